# revision 1
# baseline (speedup 1.0000x reference)
"""Trainium2 Bass kernel for ComplexGatingNetwork MoE routing.

Computes, for x = x_real + i*x_imag with x in [B=4, S=2048, D=2048]:
    amp   = |x|            (hypot)
    phase = angle(x)       (atan2, via the half-angle identity)
    scores = [amp, phase] @ W + b          -> [B, S, 64]
    top2 of softmax(scores), renormalized  -> probs [B,S,2], idx [B,S,2]

Math used on device (per element):
    s   = xr^2 + xi^2                       (fused custom DVE op)
    r   = sqrt(s + 1e-30)                   (ACT Sqrt; bias guards s == 0)
    den = r * (1 + 1e-7) + xr               (scalar_tensor_tensor; the 1e-7
                                             keeps den > 0 when r + xr would
                                             cancel to exactly 0)
    q   = xi / den                          (recip_approx_fast + multiply)
    phase = 2 * atan(q)                     (ACT Arctan; the *2 is folded
                                             into the phase weights on host)
    renormalized top-2 softmax == sigmoid(s_top1 - s_top2)

Sharding: token-parallel. Host transposes x to [D, tokens] so that the
contraction dim lands on SBUF partitions, and shards 8192 tokens across the
8 NeuronCores (1024 tokens each). The router weight is replicated.
"""

import numpy as np

import concourse.bass as bass
import concourse.bacc as bacc
import concourse.mybir as mybir
from concourse.tile import TileContext
from concourse import bass_utils

AF = mybir.ActivationFunctionType
ALU = mybir.AluOpType
F32 = mybir.dt.float32
F32R = mybir.dt.float32r
I32 = mybir.dt.int32
U32 = mybir.dt.uint32

B, S, D = 4, 2048, 2048
E = 64
TOPK = 2
N_CORES = 8
TOKENS = B * S                   # 8192
TPC = TOKENS // N_CORES          # tokens per core: 1024
KT = D // 128                    # contraction k-tiles: 16
NPAIR = KT // 2                  # k-pair units: 8
NTT = TPC // 128                 # token tiles per core: 8
HALF = 512                       # tokens per PSUM scores bank


def _register_custom(name, spec):
    from concourse import dve_ops
    from concourse.dve_spec import lower, _has_src1
    from concourse.dve_uop import DveOpSpec

    for op in dve_ops.OPS:
        if op.name == name:
            return op
    shas = {}
    for ver in ("v3", "v4"):
        uops = lower(spec, ver=ver)
        shas[ver] = DveOpSpec(name=name, opcode=0, uops=uops,
                              rd1_en=_has_src1(spec)).sha(ver)
    op = dve_ops.DveOp(name, spec, subdim=False, uops_sha=shas)
    dve_ops.OPS.append(op)
    dve_ops.CUSTOM_DVE_SPECS[name] = spec
    dve_ops._SUB_OPCODE_FOR_NAME[name] = (
        dve_ops._CUSTOM_DVE_ROW_BASE + len(dve_ops.OPS) - 1)
    return op


def _make_ops():
    import math
    from concourse.dve_spec import Spec, Src0, Src1, Zero, C0, sq, select

    sqsum = _register_custom(
        "SQSUM_ANT_KERNEL",
        Spec(body=sq(Src0) + sq(Src1),
             reference=lambda in0, in1: in0 * in0 + in1 * in1))
    # d = sign(xr)*r + xr  (Src0 = r, Src1 = xr).  |d| >= r > 0 always.
    signden = _register_custom(
        "SIGNDEN_ANT_KERNEL",
        Spec(body=select(Src1 >= Zero, Src0, Zero - Src0) + Src1,
             reference=lambda in0, in1: np.where(in1 >= 0, in0, -in0) + in1))
    # f = at              if xr >= 0
    #     at - pi/2*sg(at) if xr < 0      (Src0 = at, Src1 = xr)
    hpi = math.pi / 2
    qfix = _register_custom(
        "QFIX_ANT_KERNEL",
        Spec(body=select(Src1 >= Zero, Src0,
                         Src0 - select(Src0 >= Zero, C0, Zero - C0)),
             reference=lambda in0, in1, s0: np.where(
                 in1 >= 0, in0,
                 in0 - np.where(in0 >= 0, s0, -s0)).astype(np.float32)))
    return sqsum, signden, qfix, hpi


SQSUM, SIGNDEN, QFIX, HALF_PI = _make_ops()


def _act_recip(nc, out, in_):
    ins = [nc.scalar.lower_ap(in_),
           mybir.ImmediateValue(dtype=mybir.dt.float32, value=0.0),
           mybir.ImmediateValue(dtype=mybir.dt.float32, value=1.0),
           mybir.ImmediateValue(dtype=mybir.dt.float32, value=0.0)]
    return nc.scalar.add_instruction(
        mybir.InstActivation(
            name=nc.get_next_instruction_name(),
            func=AF.Reciprocal, ins=ins,
            outs=[nc.scalar.lower_ap(out)]))


def _build():
    nc = bacc.Bacc(None, target_bir_lowering=False, debug=False)

    xrT = nc.dram_tensor("xrT", [D, TPC], F32, kind="ExternalInput")
    xiT = nc.dram_tensor("xiT", [D, TPC], F32, kind="ExternalInput")
    wa = nc.dram_tensor("wa", [D, E], F32, kind="ExternalInput")
    wp = nc.dram_tensor("wp", [D, E], F32, kind="ExternalInput")
    bvec = nc.dram_tensor("bvec", [1, E], F32, kind="ExternalInput")
    onesv = nc.dram_tensor("onesv", [1, HALF], F32, kind="ExternalInput")
    ident = nc.dram_tensor("ident", [E, E], F32, kind="ExternalInput")

    o_probs = nc.dram_tensor("o_probs", [128, NTT * 2], F32,
                             kind="ExternalOutput")
    o_idx = nc.dram_tensor("o_idx", [128, NTT * 2], I32,
                           kind="ExternalOutput")

    xrT_v = xrT.ap().rearrange("(u j p) t -> u p j t", u=NPAIR, j=2, p=128)
    xiT_v = xiT.ap().rearrange("(u j p) t -> u p j t", u=NPAIR, j=2, p=128)
    wa_v = wa.ap().rearrange("(k p) e -> p k e", p=128)
    wp_v = wp.ap().rearrange("(k p) e -> p k e", p=128)

    with TileContext(nc) as tc:
        with tc.tile_pool(name="wpool", bufs=1) as wpool, \
             tc.tile_pool(name="inp", bufs=2) as inp, \
             tc.tile_pool(name="mid", bufs=2) as mid, \
             tc.tile_pool(name="qbuf", bufs=5) as qbuf, \
             tc.tile_pool(name="thbuf", bufs=2) as thbuf, \
             tc.tile_pool(name="small", bufs=1) as small, \
             tc.tile_pool(name="pscore", bufs=2, space="PSUM") as pscore, \
             tc.tile_pool(name="ptrans", bufs=2, space="PSUM") as ptrans:

            pre_xr = inp.tile([128, 2, TPC], F32, tag="xr", bufs=4)
            nc.sync.dma_start(pre_xr[:], xrT_v[0])
            pre_xi = inp.tile([128, 2, TPC], F32, tag="xi", bufs=3)
            nc.sync.dma_start(pre_xi[:], xiT_v[0])
            wa_sb = wpool.tile([128, KT, E], F32)
            nc.sync.dma_start(wa_sb[:], wa_v)
            wp_sb = wpool.tile([128, KT, E], F32)
            nc.sync.dma_start(wp_sb[:], wp_v)
            b_sb = wpool.tile([1, E], F32)
            nc.sync.dma_start(b_sb[:], bvec.ap())
            id_sb = wpool.tile([E, E], F32)
            nc.sync.dma_start(id_sb[:], ident.ap())
            ones_sb = wpool.tile([1, HALF], F32)
            nc.sync.dma_start(ones_sb[:], onesv.ap())
            sqrt_bias = wpool.tile([128, 1], F32)
            nc.vector.memset(sqrt_bias[:], 1e-30)

            ps = [pscore.tile([128, HALF], F32, name=f"ps{h}") for h in range(2)]

            # Two groups of 4 k-pair units.  Within a group: phase A
            # (sqrt-table ACT work) for all 4 units, then phase B
            # (arctan-table ACT work).  Keeps xr/q live-sets bounded while
            # paying only 4 ACT table loads total.  Score matmuls are
            # column-tiled: even k-tiles accumulate into PSUM partitions
            # 0-63, odd k-tiles into 64-127 (concurrent in the PE array);
            # the two halves are summed after accumulation.
            GRP = 2
            NG = NPAIR // GRP
            sqrt_insts = [[] for _ in range(NG)]
            recip_insts = [[] for _ in range(NG)]
            atan_insts = [[] for _ in range(NG)]
            for g in range(NG):
                xr_tiles = {}
                xi_tiles = {}
                den_tiles = {}
                q_tiles = {}
                # ---- phase A: load, s, r=sqrt, den + amp matmuls ----
                for u in range(g * GRP, (g + 1) * GRP):
                    if u == 0:
                        xr_u, xi_u = pre_xr, pre_xi
                    else:
                        xr_u = inp.tile([128, 2, TPC], F32, tag="xr", bufs=4)
                        nc.sync.dma_start(xr_u[:], xrT_v[u])
                        xi_u = inp.tile([128, 2, TPC], F32, tag="xi", bufs=3)
                        nc.sync.dma_start(xi_u[:], xiT_v[u])
                    xr_tiles[u] = xr_u
                    xi_tiles[u] = xi_u

                    r_u = mid.tile([128, 2, TPC], F32, tag="r")
                    s_u = mid.tile([128, 2, TPC], F32, tag="s")
                    nc.vector._custom_dve(SQSUM, out=s_u[:], in0=xr_u[:],
                                          in1=xi_u[:])
                    sq_i = nc.scalar.activation(r_u[:], s_u[:], AF.Sqrt,
                                                bias=sqrt_bias[:])
                    sqrt_insts[g].append(sq_i)
                    den_u = mid.tile([128, 2, TPC], F32, tag="den", bufs=3)
                    nc.vector._custom_dve(SIGNDEN, out=den_u[:], in0=r_u[:],
                                          in1=xr_u[:])
                    den_tiles[u] = den_u

                    for h in range(2):
                        hs = slice(h * HALF, (h + 1) * HALF)
                        for half32 in range(2):
                            es = slice(32 * half32, 32 * half32 + 32)
                            nc.tensor.matmul(
                                ps[h][32 * half32:32 * half32 + 32, :],
                                wa_sb[:, 2 * u, es], r_u[:, 0, hs],
                                start=(u == 0), stop=False,
                                tile_position=(0, 32 * half32))
                            nc.tensor.matmul(
                                ps[h][64 + 32 * half32:96 + 32 * half32, :],
                                wa_sb[:, 2 * u + 1, es], r_u[:, 1, hs],
                                start=(u == 0), stop=False,
                                tile_position=(0, 64 + 32 * half32))

                # ---- phase B: w = 1/den (ACT), q = xi*w (DVE) ----
                for u in range(g * GRP, (g + 1) * GRP):
                    w_u = mid.tile([128, 2, TPC], F32, tag="w")
                    rc_i = _act_recip(nc, w_u[:], den_tiles[u][:])
                    recip_insts[g].append(rc_i)
                    q_u = qbuf.tile([128, 2, TPC], F32, tag="q", bufs=4)
                    q_tiles[u] = q_u
                    nc.vector.tensor_tensor(out=q_u[:], in0=xi_tiles[u][:],
                                            in1=w_u[:], op=ALU.mult)

                # ---- phase C: atan, quadrant fix + phase matmuls ----
                for u in range(g * GRP, (g + 1) * GRP):
                    at_u = thbuf.tile([128, 2, TPC], F32, tag="at")
                    at_i = nc.scalar.activation(at_u[:], q_tiles[u][:],
                                                AF.Arctan)
                    atan_insts[g].append(at_i)
                    th_u = at_u
                    nc.vector._custom_dve(QFIX, out=th_u[:], in0=at_u[:],
                                          in1=xr_tiles[u][:], s0=HALF_PI)
                    last = (u == NPAIR - 1)
                    for h in range(2):
                        hs = slice(h * HALF, (h + 1) * HALF)
                        for half32 in range(2):
                            es = slice(32 * half32, 32 * half32 + 32)
                            nc.tensor.matmul(
                                ps[h][32 * half32:32 * half32 + 32, :],
                                wp_sb[:, 2 * u, es], th_u[:, 0, hs],
                                start=False, stop=False,
                                tile_position=(0, 32 * half32))
                            nc.tensor.matmul(
                                ps[h][64 + 32 * half32:96 + 32 * half32, :],
                                wp_sb[:, 2 * u + 1, es], th_u[:, 1, hs],
                                start=False, stop=last,
                                tile_position=(0, 64 + 32 * half32))

            # ACT-stream table phase order within and across groups:
            # sqrt(g) < recip(g) < atan(g) < sqrt(g+1)
            from concourse.tile import add_dep_helper
            for g in range(NG):
                for a in recip_insts[g]:
                    for s_i in sqrt_insts[g]:
                        add_dep_helper(a.ins, s_i.ins, sync=False,
                                       reason="ACT table phase order")
                for a in atan_insts[g]:
                    for s_i in recip_insts[g]:
                        add_dep_helper(a.ins, s_i.ins, sync=False,
                                       reason="ACT table phase order")
                if g + 1 < NG:
                    for s2 in sqrt_insts[g + 1]:
                        for a in atan_insts[g]:
                            add_dep_helper(s2.ins, a.ins, sync=False,
                                           reason="ACT table phase order")

            # ---- bias (even-column groups) ----
            for h in range(2):
                for half32 in range(2):
                    nc.tensor.matmul(
                        ps[h][32 * half32:32 * half32 + 32, :],
                        b_sb[:, 32 * half32:32 * half32 + 32], ones_sb[:],
                        start=False, stop=True,
                        tile_position=(0, 32 * half32))

            # ---- scores fixup: transpose S^T -> [tokens, E], top-2 ----
            probs_acc = small.tile([128, NTT, 2], F32)
            idx_acc = small.tile([128, NTT, 2], I32)
            for h in range(2):
                s_ev = small.tile([E, HALF], F32, tag="s_ev")
                nc.scalar.copy(s_ev[:], ps[h][0:64, :])
                s_sb = small.tile([E, HALF], F32, tag="s_sb")
                nc.vector.tensor_tensor(out=s_sb[:], in0=s_ev[:],
                                        in1=ps[h][64:128, :], op=ALU.add)
                for c in range(HALF // 128):
                    n = h * (HALF // 128) + c
                    ps_t = ptrans.tile([128, E], F32, tag="pst")
                    nc.tensor.transpose(ps_t[:], s_sb[:, c * 128:(c + 1) * 128],
                                        id_sb[:])
                    sc_t = small.tile([128, E], F32, tag="sc_t")
                    nc.scalar.copy(sc_t[:], ps_t[:])
                    vals = small.tile([128, 8], F32, tag="vals")
                    idxs = small.tile([128, 8], U32, tag="idxs")
                    nc.vector.max_with_indices(vals[:], idxs[:], sc_t[:])
                    d12 = small.tile([128, 2], F32, tag="d12")
                    nc.vector.tensor_sub(d12[:, 0:1], vals[:, 0:1], vals[:, 1:2])
                    nc.vector.tensor_sub(d12[:, 1:2], vals[:, 1:2], vals[:, 0:1])
                    nc.scalar.activation(probs_acc[:, n, :], d12[:], AF.Sigmoid)
                    nc.vector.tensor_copy(idx_acc[:, n, :], idxs[:, 0:2])

            nc.sync.dma_start(
                o_probs.ap().rearrange("p (n k) -> p n k", k=2), probs_acc[:])
            nc.sync.dma_start(
                o_idx.ap().rearrange("p (n k) -> p n k", k=2), idx_acc[:])

    nc.compile()
    return nc


_NC_CACHE = None


def _get_nc():
    global _NC_CACHE
    if _NC_CACHE is None:
        _NC_CACHE = _build()
    return _NC_CACHE


def _make_in_maps(inputs):
    x_real = np.asarray(inputs["x_real"])
    x_imag = np.asarray(inputs["x_imag"])
    W = np.asarray(inputs["W"], dtype=np.float32)
    b = np.asarray(inputs["b"], dtype=np.float32)

    xr = x_real.reshape(TOKENS, D)
    xi = x_imag.reshape(TOKENS, D)

    wa = np.ascontiguousarray(W[:D]).astype(np.float32)
    wp = np.ascontiguousarray(2.0 * W[D:]).astype(np.float32)
    bvec = b.reshape(1, E)
    ident = np.eye(E, dtype=np.float32)
    ones = np.ones((1, HALF), np.float32)

    in_maps = []
    for c in range(N_CORES):
        sl = slice(c * TPC, (c + 1) * TPC)
        in_maps.append({
            "xrT": np.ascontiguousarray(xr[sl].T),
            "xiT": np.ascontiguousarray(xi[sl].T),
            "wa": wa,
            "wp": wp,
            "bvec": bvec,
            "onesv": ones,
            "ident": ident,
        })
    return in_maps


def kernel(x_real, x_imag, W, b):
    in_maps = _make_in_maps(
        {"x_real": x_real, "x_imag": x_imag, "W": W, "b": b})
    nc = _get_nc()
    res = bass_utils.run_bass_kernel_spmd(nc, in_maps,
                                          core_ids=list(range(N_CORES)))

    probs = np.empty((TOKENS, TOPK), np.float32)
    idx = np.empty((TOKENS, TOPK), np.int32)
    for c in range(N_CORES):
        out = res.results[c]
        p = out["o_probs"].reshape(128, NTT, 2).transpose(1, 0, 2)
        i = out["o_idx"].reshape(128, NTT, 2).transpose(1, 0, 2)
        probs[c * TPC:(c + 1) * TPC] = p.reshape(TPC, 2)
        idx[c * TPC:(c + 1) * TPC] = i.reshape(TPC, 2)

    return (probs.reshape(B, S, TOPK), idx.reshape(B, S, TOPK))

